# revision 5
# baseline (speedup 1.0000x reference)
"""Trainium2 Bass kernel for nn_ConstraintProblog (SDD circuit evaluation).

Strategy
--------
Data parallel on the batch axis B=4096 across 8 NeuronCores (512 each).
The circuit structure (int32 index arrays) is replicated and treated as
compile-time data: on the host we
  1. dead-code-eliminate everything not reachable from the P=8 query
     nodes (the random DAG keeps only ~86/4096 internal nodes live),
  2. remap leaf literals directly onto an 80-row P / (1-P) probability
     table (so leaves are never materialized),
  3. level-schedule the surviving nodes (children strictly earlier).

On device, everything stays in SBUF. A value table of [row, batch=512]
f32 chunks ([128, 512] each) holds the softmax rows + one 32-aligned
row block per level of live nodes (compute engines can only address
partition ranges starting at multiples of 32).  Per level, the (gate,
left, right) operand rows are gathered with one-hot fp32 matmuls on the
TensorEngine (verified bit-exact on HW), combined on the VectorEngine
as b + g*(a-b), and written back into that level's table rows.
Softmax (temperature-calibrated) and the final logit/sigmoid
calibration also run on device; only int-index preprocessing happens on
the host.
"""
import sys

if "/opt/trn_rl_repo" not in sys.path:
    sys.path.insert(0, "/opt/trn_rl_repo")

import numpy as np

# Problem shapes (hardcoded per spec).
V, D, B = 4, 10, 4096
L, N, P = 1024, 4096, 8
EPS = 1e-12
NCORES = 8
BL = B // NCORES  # 512 batch elements per core

NEG_OFF = 64                # 1-P table starts at this row (32-aligned, >= V*D)
LEAF_SPAN = 128             # leaf region rows 0..127; level blocks start at 128
MAX_GROUP = 32              # one 32-aligned row slot per level group
UPPER_CLIP = float(np.nextafter(np.float32(1.0), np.float32(0.0)))

_CACHE = {}


def _schedule(leaf_var, leaf_cls, leaf_neg, gate, node_left, node_right):
    """Host-side structure preprocessing. Returns the level schedule."""
    leaf_var = np.asarray(leaf_var, np.int64)
    leaf_cls = np.asarray(leaf_cls, np.int64)
    leaf_neg = np.asarray(leaf_neg, np.int64)
    gate = np.asarray(gate, np.int64)
    node_left = np.asarray(node_left, np.int64)
    node_right = np.asarray(node_right, np.int64)

    assert V * D <= NEG_OFF and NEG_OFF + V * D <= LEAF_SPAN
    # leaf reference -> row in the P/(1-P) table
    leaf_row = leaf_var * D + leaf_cls + NEG_OFF * leaf_neg  # [L]

    # Reachability from the last P nodes (the only ones read).
    reach = np.zeros(N, bool)
    reach[N - P:] = True
    for i in range(N - 1, -1, -1):
        if reach[i]:
            for ch in (gate[i], node_left[i], node_right[i]):
                if ch >= L:
                    reach[ch - L] = True

    live = np.flatnonzero(reach)

    # Levels (leaves are level 0).
    lvl = np.zeros(N, np.int64)
    for i in live:
        m = 0
        for ch in (node_left[i], node_right[i], gate[i]):
            if ch >= L:
                m = max(m, lvl[ch - L])
        lvl[i] = m + 1

    order = live[np.argsort(lvl[live], kind="stable")]
    row_of = np.full(N, -1, np.int64)

    # Assign each group of <= 32 same-level nodes its own 32-row slot.
    node_groups = []
    pos, r = 0, LEAF_SPAN
    while pos < len(order):
        this_lvl = lvl[order[pos]]
        end = pos
        while (end < len(order) and lvl[order[end]] == this_lvl
               and end - pos < MAX_GROUP):
            end += 1
        nodes = order[pos:end]
        node_groups.append((nodes, r))
        row_of[nodes] = r + np.arange(len(nodes))
        r += MAX_GROUP
        pos = end
    nrows = r

    def ref_row(ch):
        return leaf_row[ch] if ch < L else row_of[ch - L]

    groups = []
    for nodes, r0 in node_groups:
        g = np.array([ref_row(gate[i]) for i in nodes])
        a = np.array([ref_row(node_left[i]) for i in nodes])
        b = np.array([ref_row(node_right[i]) for i in nodes])
        groups.append((g, a, b, r0))

    out_rows = np.array([row_of[i] for i in range(N - P, N)])
    assert (out_rows >= 0).all()
    return groups, out_rows, nrows


def _build(groups, out_rows, nrows):
    """Build + compile the Bass program specialized to the schedule."""
    import concourse.bacc as bacc
    import concourse.tile as tile
    from concourse import mybir

    f32 = mybir.dt.float32
    AF = mybir.ActivationFunctionType
    OP = mybir.AluOpType

    CH = (nrows + 127) // 128
    assert CH * 128 <= 2048, f"live circuit too large: {nrows} rows"

    # --- selection (one-hot) weight matrices, one per table chunk ---
    # per group: 96 columns (G at +0, A at +32, B at +64), then P output cols
    GW = 3 * MAX_GROUP
    ncols = GW * len(groups) + P
    sel = [np.zeros((128, ncols), np.float32) for _ in range(CH)]
    group_cols = []
    for gi, (g, a, b, _) in enumerate(groups):
        col = GW * gi
        group_cols.append(col)
        for j, rows in enumerate((g, a, b)):
            for k, r in enumerate(rows):
                sel[r // 128][r % 128, col + j * MAX_GROUP + k] = 1.0
    qcol = GW * len(groups)
    for k, r in enumerate(out_rows):
        sel[r // 128][r % 128, qcol + k] = 1.0

    # constant matrices for the softmax cross-partition reduce/broadcast
    oneblk = np.zeros((V * D, V), np.float32)   # lhsT: sums groups of D rows
    ebcast = np.zeros((V, V * D), np.float32)   # lhsT: broadcasts V -> V*D rows
    for p in range(V * D):
        oneblk[p, p // D] = 1.0
        ebcast[p // D, p] = 1.0

    nc = bacc.Bacc("TRN2", target_bir_lowering=False, debug=False,
                   num_devices=NCORES)

    imgs_d = nc.dram_tensor("imgsT", [V * D, BL], f32, kind="ExternalInput").ap()
    ct_d = nc.dram_tensor("ctemp", [V, 128], f32, kind="ExternalInput").ap()
    pt_d = nc.dram_tensor("ptemp", [P, 128], f32, kind="ExternalInput").ap()
    ob_d = nc.dram_tensor("oneblk", [V * D, V], f32, kind="ExternalInput").ap()
    eb_d = nc.dram_tensor("ebcast", [V, V * D], f32, kind="ExternalInput").ap()
    sel_d = [nc.dram_tensor(f"sel{c}", [128, ncols], f32, kind="ExternalInput").ap()
             for c in range(CH)]
    out_d = nc.dram_tensor("out", [P, BL], f32, kind="ExternalOutput").ap()

    with tile.TileContext(nc) as tc:
        with tc.tile_pool(name="const", bufs=1) as cpool, \
             tc.tile_pool(name="tab", bufs=1) as tpool, \
             tc.tile_pool(name="work", bufs=2) as wpool, \
             tc.tile_pool(name="ps", bufs=2, space="PSUM") as pspool, \
             tc.tile_pool(name="psw", bufs=1, space="PSUM") as pswpool:

            X = cpool.tile([V * D, BL], f32, tag="X")
            OB = cpool.tile([V * D, V], f32, tag="OB")
            EB = cpool.tile([V, V * D], f32, tag="EB")
            CT = cpool.tile([V, 128], f32, tag="CT")
            PT = cpool.tile([P, 128], f32, tag="PT")
            SEL = [cpool.tile([128, ncols], f32, tag=f"SEL{c}", name=f"SEL{c}")
                   for c in range(CH)]
            T = [tpool.tile([128, BL], f32, tag=f"T{c}", name=f"T{c}")
                 for c in range(CH)]

            nc.sync.dma_start(X[:], imgs_d[:])
            nc.sync.dma_start(OB[:], ob_d[:])
            nc.sync.dma_start(EB[:], eb_d[:])
            nc.sync.dma_start(CT[:], ct_d[:])
            nc.sync.dma_start(PT[:], pt_d[:])
            for c in range(CH):
                nc.sync.dma_start(SEL[c][:], sel_d[c][:])
                nc.any.memset(T[c][:], 0.0)

            # ---- temperature scale 1/(relu(t)+eps), broadcast V -> V*D ----
            rt = wpool.tile([V, 1], f32, tag="rt")
            nc.scalar.activation(rt[:], CT[:, 0:1], AF.Relu)
            nc.vector.tensor_scalar_add(rt[:], rt[:], EPS)
            rct = wpool.tile([V, 1], f32, tag="rct")
            nc.vector.reciprocal(rct[:], rt[:])
            psA = pswpool.tile([V * D, 1], f32, tag="psA")
            nc.tensor.matmul(psA[:], EB[:], rct[:], start=True, stop=True)
            sct = wpool.tile([V * D, 1], f32, tag="sct")
            nc.vector.tensor_copy(sct[:], psA[:])

            # ---- softmax: exp(x/t) then divide by per-(v,b) sum ----
            EXPX = wpool.tile([V * D, BL], f32, tag="EXPX")
            nc.scalar.activation(EXPX[:], X[:], AF.Exp, scale=sct[:])
            psS = pswpool.tile([V, BL], f32, tag="psS")
            nc.tensor.matmul(psS[:], OB[:], EXPX[:], start=True, stop=True)
            rden = wpool.tile([V, BL], f32, tag="rden")
            nc.vector.reciprocal(rden[:], psS[:])
            psB = pswpool.tile([V * D, BL], f32, tag="psB")
            nc.tensor.matmul(psB[:], EB[:], rden[:], start=True, stop=True)
            # P rows then (1-P) rows of the table
            nc.vector.tensor_mul(T[0][0:V * D, :], EXPX[:], psB[:])
            nc.vector.tensor_scalar(T[0][NEG_OFF:NEG_OFF + V * D, :],
                                    T[0][0:V * D, :],
                                    -1.0, 1.0, OP.mult, OP.add)

            # ---- level-ordered circuit evaluation ----
            for gi, (g, a, b, row0) in enumerate(groups):
                K = len(g)
                co = group_cols[gi]
                src_chunks = sorted({int(r) // 128
                                     for r in np.concatenate([g, a, b])})
                psG = pspool.tile([GW, BL], f32, tag="gab", name=f"gab{gi}")
                for i, c in enumerate(src_chunks):
                    nc.tensor.matmul(psG[:], SEL[c][:, co:co + GW], T[c][:],
                                     start=(i == 0),
                                     stop=(i == len(src_chunks) - 1))
                bsb = wpool.tile([MAX_GROUP, BL], f32, tag="bsb", name=f"bsb{gi}")
                nc.vector.tensor_copy(bsb[0:K, :], psG[64:64 + K, :])
                dsb = wpool.tile([MAX_GROUP, BL], f32, tag="dsb", name=f"dsb{gi}")
                nc.vector.tensor_sub(dsb[0:K, :], psG[32:32 + K, :], bsb[0:K, :])
                msb = wpool.tile([MAX_GROUP, BL], f32, tag="msb", name=f"msb{gi}")
                nc.vector.tensor_mul(msb[0:K, :], psG[0:K, :], dsb[0:K, :])
                c, rp = row0 // 128, row0 % 128
                nc.vector.tensor_add(T[c][rp:rp + K, :],
                                     msb[0:K, :], bsb[0:K, :])

            # ---- final gather + output calibration ----
            psQ = pswpool.tile([P, BL], f32, tag="psQ")
            qchunks = sorted({int(r) // 128 for r in out_rows})
            for i, c in enumerate(qchunks):
                nc.tensor.matmul(psQ[:], SEL[c][:, qcol:qcol + P], T[c][:],
                                 start=(i == 0), stop=(i == len(qchunks) - 1))
            qs = wpool.tile([P, BL], f32, tag="qs")
            nc.vector.tensor_scalar(qs[:], psQ[:], EPS, UPPER_CLIP,
                                    OP.max, OP.min)
            om = wpool.tile([P, BL], f32, tag="om")
            nc.vector.tensor_scalar(om[:], qs[:], -1.0, 1.0, OP.mult, OP.add)
            ro = wpool.tile([P, BL], f32, tag="ro")
            nc.vector.reciprocal(ro[:], om[:])
            rr = wpool.tile([P, BL], f32, tag="rr")
            nc.vector.tensor_mul(rr[:], qs[:], ro[:])
            lg = wpool.tile([P, BL], f32, tag="lg")
            nc.scalar.activation(lg[:], rr[:], AF.Ln)

            prt = wpool.tile([P, 1], f32, tag="prt")
            nc.scalar.activation(prt[:], PT[:, 0:1], AF.Relu)
            nc.vector.tensor_scalar_add(prt[:], prt[:], EPS)
            rpt = wpool.tile([P, 1], f32, tag="rpt")
            nc.vector.reciprocal(rpt[:], prt[:])

            ov = wpool.tile([P, BL], f32, tag="ov")
            nc.scalar.activation(ov[:], lg[:], AF.Sigmoid, scale=rpt[:])
            nc.sync.dma_start(out_d[:], ov[:])

    nc.compile()
    consts = {"oneblk": oneblk, "ebcast": ebcast}
    for c in range(CH):
        consts[f"sel{c}"] = sel[c]
    return nc, consts


def kernel(imgs, classes_temp_logits, props_temp_logits,
           leaf_var, leaf_cls, leaf_neg, gate, node_left, node_right):
    from concourse import bass_utils

    imgs = np.asarray(imgs, np.float32)
    ct = np.repeat(np.asarray(classes_temp_logits, np.float32).reshape(V, 1),
                   128, axis=1)
    pt = np.repeat(np.asarray(props_temp_logits, np.float32).reshape(P, 1),
                   128, axis=1)

    key = tuple(np.concatenate([
        np.asarray(x, np.int64).ravel()
        for x in (leaf_var, leaf_cls, leaf_neg, gate, node_left, node_right)
    ]).tolist())
    khash = hash(key)
    if khash not in _CACHE:
        groups, out_rows, nrows = _schedule(leaf_var, leaf_cls, leaf_neg,
                                            gate, node_left, node_right)
        _CACHE[khash] = _build(groups, out_rows, nrows)
    nc, consts = _CACHE[khash]

    in_maps = []
    for k in range(NCORES):
        sl = imgs[:, k * BL:(k + 1) * BL, :]                 # [V, BL, D]
        imgsT = np.ascontiguousarray(
            sl.transpose(0, 2, 1).reshape(V * D, BL))        # [(v d), b]
        m = {"imgsT": imgsT, "ctemp": ct, "ptemp": pt}
        m.update(consts)
        in_maps.append(m)

    res = bass_utils.run_bass_kernel_spmd(nc, in_maps,
                                          core_ids=list(range(NCORES)))
    out = np.empty((P, B), np.float32)
    for k in range(NCORES):
        out[:, k * BL:(k + 1) * BL] = res.results[k]["out"]
    return out
